# revision 18
# baseline (speedup 1.0000x reference)
"""KAN layer (B=8192, IN_F=OUT_F=1024, GRID=5) on 8 Trainium2 cores.

Math: Y[b,o] = W0[o]*silu(x) + W1[o]*spline_o(clip(x,-1,1)) + b[o], x = X[b,o]
(idx_in = arange(O) % IN_F is the identity here since O == IN_F).

The degree-1 B-spline on the uniform 5-knot grid over [-1,1] is rewritten in
the relu basis: spline(xc) = c0 + m0*(xc+1) + sum_j (m_j - m_{j-1})*relu(xc - s_j)
with slopes m_g = 2*(c_{g+1}-c_g) and interior knots s_j in {-0.5, 0, 0.5}.
Folding W1 and b gives  Y^T[o,:] = W0*silu + B'*xc + G1*r1 + G2*r2 + G3*r3 + A'.

Layout: edges on SBUF partitions (X pre-transposed on host), batch on the free
dim, data-parallel over batch across the 8 cores.  The per-edge weighted sum of
the 5 feature maps runs on TensorE as 5 diagonal-stationary matmuls (silu in
fp32r, the four spline features in fp16) accumulating in PSUM; ScalarE
evacuates PSUM adding the per-edge bias A'.  Diagonal stationaries are built
on-device (identity * per-partition weight).  DMA: per-block input loads on the
Sync HWDGE queue, output stores on GpSimd SWDGE — ScalarE issues no DMAs.
"""
import sys

for _p in ("/root/.axon_site", "/root/.axon_site/_ro/trn_rl_repo", "/root/.axon_site/_ro/pypackages"):
    if _p not in sys.path:
        sys.path.append(_p)

import numpy as np

import concourse.bacc as bacc
import concourse.tile as tile
from concourse import mybir
from concourse.bass_utils import run_bass_kernel_spmd

B, IN_F, OUT_F, GRID = 8192, 1024, 1024, 5
N_CORES = 8
B_SHARD = B // N_CORES          # 1024 batch rows per core
EB = OUT_F // 128               # 8 edge blocks
NF = 5                          # features: silu, xc, r1, r2, r3
CHUNK = 512                     # one PSUM bank of fp32

# cpack: fp32 [128, 8] = per-block A' bias columns.  dpack16: fp16 diagonal
# stationaries for the 4 spline features x 8 blocks.  dpackr: fp32r diagonal
# stationaries for the silu feature x 8 blocks.
AOFF, CCOLS = 0, 8

_nc_cache = None


def _build():
    f32 = mybir.dt.float32
    f32r = mybir.dt.float32r
    f16 = mybir.dt.float16
    AF = mybir.ActivationFunctionType
    OP = mybir.AluOpType
    nc = bacc.Bacc("TRN2", target_bir_lowering=False, debug=False)
    xt = nc.dram_tensor("xt", [OUT_F, B_SHARD], f32, kind="ExternalInput").ap()
    cpack = nc.dram_tensor("cpack", [128, CCOLS], f32, kind="ExternalInput").ap()
    dpack16 = nc.dram_tensor("dpack16", [128, EB * 4 * 128], f16, kind="ExternalInput").ap()
    dpackr = nc.dram_tensor("dpackr", [128, EB * 128], f32r, kind="ExternalInput").ap()
    yt = nc.dram_tensor("yt", [OUT_F, B_SHARD], f16, kind="ExternalOutput").ap()

    xt3 = xt.rearrange("(n p) d -> p n d", p=128)   # [128, EB, B_SHARD]
    yt3 = yt.rearrange("(n p) d -> p n d", p=128)

    with tile.TileContext(nc) as tc:
        with tc.tile_pool(name="const", bufs=1) as const_pool, \
             tc.tile_pool(name="xin", bufs=4) as xin_pool, \
             tc.tile_pool(name="feat", bufs=2) as feat_pool, \
             tc.tile_pool(name="yout", bufs=2) as yout_pool, \
             tc.tile_pool(name="ps", bufs=3, space="PSUM") as psum_pool, \
             tc.tile_pool(name="pswarm", bufs=1, space="PSUM") as warm_pool:
            cp = const_pool.tile([128, CCOLS], f32)
            nc.sync.dma_start(cp[:], cpack[:, :])
            dspl = const_pool.tile([128, EB * 4 * 128], f16)
            nc.gpsimd.dma_start(dspl[:, 0:16 * 128], dpack16[:, 0:16 * 128])
            dsilu = const_pool.tile([128, EB * 128], f32r)
            nc.gpsimd.dma_start(dsilu[:], dpackr[:, :])
            nc.gpsimd.dma_start(dspl[:, 16 * 128:], dpack16[:, 16 * 128:])

            # HAM warm-up: ~4.5us of dummy matmuls on uninitialized SBUF so
            # the PE clock-gate opens before the first real matmul arrives
            scratch = const_pool.tile([128, CHUNK], f16)
            nc.vector.memset(scratch[:], 0.0)
            ps_warm = warm_pool.tile([128, CHUNK], f32, tag="pswarm", name="pswarm")
            for _ in range(9):
                nc.tensor.matmul(ps_warm[:], scratch[:, 0:128], scratch[:],
                                 start=True, stop=True, skip_group_check=True)


            def feature_ops(xv, n, tagsuf):
                """xv: [128, n, B_SHARD] input view -> 5 feature tiles."""
                silu_t = feat_pool.tile([128, n, B_SHARD], f32r, tag="silu" + tagsuf,
                                        name=f"silu{tagsuf}")
                nc.scalar.activation(silu_t[:], xv, AF.Silu)
                xc_t = feat_pool.tile([128, n, B_SHARD], f16, tag="xc" + tagsuf,
                                      name=f"xc{tagsuf}")
                nc.vector.tensor_scalar(xc_t[:], xv, 1.0, -1.0, OP.min, OP.max)
                r1_t = feat_pool.tile([128, n, B_SHARD], f16, tag="r1" + tagsuf,
                                      name=f"r1{tagsuf}")
                nc.vector.tensor_scalar(r1_t[:], xc_t[:], 0.5, 0.0, OP.add, OP.max)
                r2_t = feat_pool.tile([128, n, B_SHARD], f16, tag="r2" + tagsuf,
                                      name=f"r2{tagsuf}")
                nc.vector.tensor_scalar_max(r2_t[:], xc_t[:], 0.0)
                r3_t = feat_pool.tile([128, n, B_SHARD], f16, tag="r3" + tagsuf,
                                      name=f"r3{tagsuf}")
                nc.vector.tensor_scalar(r3_t[:], xc_t[:], -0.5, 0.0, OP.add, OP.max)
                return silu_t, xc_t, r1_t, r2_t, r3_t

            def block_matmuls(e, feats, hh, yo):
                """Build diags for block e, run the 10 matmuls, evacuate."""
                silu_t, xc_t, r1_t, r2_t, r3_t = feats
                ds = dsilu[:, e * 128:(e + 1) * 128]
                ps = psum_pool.tile([128, B_SHARD], f32, tag="ps", name=f"ps_{e}")

                # xc is ready before silu (clip is cheaper than the ACT pass),
                # so start each block's accumulation with the spline features
                # and finish with silu
                def block_chunk(ts):
                    for j, ft in enumerate((xc_t, r1_t, r2_t, r3_t)):
                        for t in ts:
                            nc.tensor.matmul(ps[:, t * CHUNK:(t + 1) * CHUNK],
                                             dspl[:, (e * 4 + j) * 128:(e * 4 + j + 1) * 128],
                                             ft[:, hh, t * CHUNK:(t + 1) * CHUNK],
                                             start=(j == 0), stop=False,
                                             skip_group_check=True)
                    for t in ts:
                        nc.tensor.matmul(ps[:, t * CHUNK:(t + 1) * CHUNK], ds,
                                         silu_t[:, hh, t * CHUNK:(t + 1) * CHUNK],
                                         start=False, stop=True, skip_group_check=True)

                if e < EB - 1:
                    block_chunk((0, 1))
                    nc.scalar.activation(yo[:, e % 2, :], ps[:], AF.Identity,
                                         bias=cp[:, AOFF + e:AOFF + e + 1], scale=1.0)
                else:
                    # last block: per-chunk pipeline on VectorE for a short tail
                    for t in range(2):
                        block_chunk((t,))
                        nc.vector.tensor_scalar_add(
                            yo[:, e % 2, t * CHUNK:(t + 1) * CHUNK],
                            ps[:, t * CHUNK:(t + 1) * CHUNK],
                            cp[:, AOFF + e:AOFF + e + 1])

            for ep in range(EB // 2):
                if ep == 0:
                    # first pair: per-block DMAs and per-block features so
                    # compute starts as soon as 512 KB has landed
                    yo = yout_pool.tile([128, 2, B_SHARD], f16, tag="yo", name="yo_p0")
                    for h in range(2):
                        x_t = xin_pool.tile([128, 1, B_SHARD], f32, tag=f"x0{h}",
                                            name=f"x0{h}")
                        nc.sync.dma_start(x_t[:], xt3[:, h:h + 1, :])
                        feats = feature_ops(x_t[:], 1, f"0{h}")
                        block_matmuls(h, feats, 0, yo)
                else:
                    x_t = xin_pool.tile([128, 2, B_SHARD], f32, tag="x",
                                        name=f"x_p{ep}")
                    nc.sync.dma_start(x_t[:], xt3[:, 2 * ep:2 * ep + 2, :])
                    feats = feature_ops(x_t[:], 2, "")
                    yo = yout_pool.tile([128, 2, B_SHARD], f16, tag="yo",
                                        name=f"yo_p{ep}")
                    for h in range(2):
                        block_matmuls(2 * ep + h, feats, h, yo)
                if ep == EB // 2 - 1:
                    # split the last stores across two queues for a short tail
                    nc.gpsimd.dma_start(yt3[:, 2 * ep:2 * ep + 1, :], yo[:, 0:1, :])
                    nc.gpsimd.dma_start(yt3[:, 2 * ep + 1:2 * ep + 2, 0:CHUNK],
                                        yo[:, 1:2, 0:CHUNK])
                    nc.sync.dma_start(yt3[:, 2 * ep + 1:2 * ep + 2, CHUNK:B_SHARD],
                                      yo[:, 1:2, CHUNK:B_SHARD])
                elif ep % 2 == 0:
                    nc.gpsimd.dma_start(yt3[:, 2 * ep:2 * ep + 2, :], yo[:])
                else:
                    nc.sync.dma_start(yt3[:, 2 * ep:2 * ep + 2, :], yo[:])
    nc.compile()
    return nc


def _host_prep(X, coeffs, W, b):
    c = coeffs.astype(np.float64)
    W = W.astype(np.float64)
    b = b.astype(np.float64)
    m = 2.0 * (c[:, 1:] - c[:, :-1])            # [O, 4] slopes per unit xc
    w1 = W[:, 1]
    aprime = w1 * (c[:, 0] + m[:, 0]) + b        # const term (incl. m0*(xc+1) fold)
    bprime = w1 * m[:, 0]
    g = w1[:, None] * (m[:, 1:] - m[:, :-1])     # [O, 3] relu weights at s=-0.5,0,0.5
    wvec = np.stack([W[:, 0], bprime, g[:, 0], g[:, 1], g[:, 2]], axis=1)  # [O, 5]

    cpack = np.zeros((128, CCOLS), dtype=np.float32)
    k = np.arange(128)
    dpack16 = np.zeros((128, EB * 4 * 128), dtype=np.float16)
    dpackr = np.zeros((128, EB * 128), dtype=np.float32)
    for e in range(EB):
        cpack[:, AOFF + e] = aprime[e * 128:(e + 1) * 128].astype(np.float32)
        dpackr[k, e * 128 + k] = wvec[e * 128 + k, 0].astype(np.float32)
        for j in range(4):
            dpack16[k, (e * 4 + j) * 128 + k] = wvec[e * 128 + k, 1 + j].astype(np.float16)
    return cpack, dpack16, dpackr


def kernel(X, coeffs, W, b):
    global _nc_cache
    if _nc_cache is None:
        _nc_cache = _build()
    nc = _nc_cache

    cpack, dpack16, dpackr = _host_prep(X, coeffs, W, b)
    in_maps = []
    for c in range(N_CORES):
        xt_shard = np.ascontiguousarray(X[c * B_SHARD:(c + 1) * B_SHARD, :].T)
        in_maps.append({"xt": xt_shard, "cpack": cpack,
                        "dpack16": dpack16, "dpackr": dpackr})

    res = run_bass_kernel_spmd(nc, in_maps, core_ids=list(range(N_CORES)))
    Y = np.empty((B, OUT_F), dtype=np.float32)
    for c in range(N_CORES):
        Y[c * B_SHARD:(c + 1) * B_SHARD, :] = res.results[c]["yt"].T.astype(np.float32)
    return Y


# revision 20
# speedup vs baseline: 1.1050x; 1.1050x over previous
"""KAN layer (B=8192, IN_F=OUT_F=1024, GRID=5) on 8 Trainium2 cores.

Math: Y[b,o] = W0[o]*silu(x) + W1[o]*spline_o(clip(x,-1,1)) + b[o], x = X[b,o]
(idx_in = arange(O) % IN_F is the identity here since O == IN_F).

The degree-1 B-spline on the uniform 5-knot grid over [-1,1] is rewritten in
the relu basis: spline(xc) = c0 + m0*(xc+1) + sum_j (m_j - m_{j-1})*relu(xc - s_j)
with slopes m_g = 2*(c_{g+1}-c_g) and interior knots s_j in {-0.5, 0, 0.5}.
Folding W1 and b gives  Y^T[o,:] = W0*silu + B'*xc + G1*r1 + G2*r2 + G3*r3 + A'.

Layout: edges on SBUF partitions (X pre-transposed on host), batch on the free
dim, data-parallel over batch across the 8 cores.  The per-edge weighted sum of
the 5 feature maps runs on TensorE as 5 diagonal-stationary matmuls (silu in
fp32r, the four spline features in fp16) accumulating in PSUM; ScalarE
evacuates PSUM adding the per-edge bias A'.  Diagonal stationaries are built
on-device (identity * per-partition weight).  DMA: per-block input loads on the
Sync HWDGE queue, output stores on GpSimd SWDGE — ScalarE issues no DMAs.
"""
import sys

for _p in ("/root/.axon_site", "/root/.axon_site/_ro/trn_rl_repo", "/root/.axon_site/_ro/pypackages"):
    if _p not in sys.path:
        sys.path.append(_p)

import numpy as np

import concourse.bacc as bacc
import concourse.tile as tile
from concourse import mybir
from concourse.bass_utils import run_bass_kernel_spmd

B, IN_F, OUT_F, GRID = 8192, 1024, 1024, 5
N_CORES = 8
B_SHARD = B // N_CORES          # 1024 batch rows per core
EB = OUT_F // 128               # 8 edge blocks
NF = 5                          # features: silu, xc, r1, r2, r3
CHUNK = 512                     # one PSUM bank of fp32

# cpack layout (fp32 columns): [0:128] identity, [128:168] wT (5 weights x 8
# blocks, feature-major per block), [168:176] A'
WOFF, AOFF, CCOLS = 128, 168, 176

_nc_cache = None


def _build():
    f32 = mybir.dt.float32
    f32r = mybir.dt.float32r
    f16 = mybir.dt.float16
    AF = mybir.ActivationFunctionType
    OP = mybir.AluOpType
    nc = bacc.Bacc("TRN2", target_bir_lowering=False, debug=False)
    xt = nc.dram_tensor("xt", [OUT_F, B_SHARD], f32, kind="ExternalInput").ap()
    cpack = nc.dram_tensor("cpack", [128, CCOLS], f32, kind="ExternalInput").ap()
    yt = nc.dram_tensor("yt", [OUT_F, B_SHARD], f16, kind="ExternalOutput").ap()

    xt3 = xt.rearrange("(n p) d -> p n d", p=128)   # [128, EB, B_SHARD]
    yt3 = yt.rearrange("(n p) d -> p n d", p=128)

    with tile.TileContext(nc) as tc:
        with tc.tile_pool(name="const", bufs=1) as const_pool, \
             tc.tile_pool(name="xin", bufs=4) as xin_pool, \
             tc.tile_pool(name="feat", bufs=3) as feat_pool, \
             tc.tile_pool(name="feat0", bufs=1) as feat0_pool, \
             tc.tile_pool(name="yout", bufs=2) as yout_pool, \
             tc.tile_pool(name="ps", bufs=3, space="PSUM") as psum_pool, \
             tc.tile_pool(name="pswarm", bufs=1, space="PSUM") as warm_pool:
            cp = const_pool.tile([128, CCOLS], f32)
            nc.sync.dma_start(cp[:], cpack[:, :])
            ident32 = cp[:, 0:128]
            wv = cp[:, WOFF:WOFF + 40]               # [128, 40] fp32 weights
            ident16 = const_pool.tile([128, 128], f16)
            nc.vector.tensor_copy(ident16[:], ident32)

            # HAM warm-up: ~4.5us of dummy matmuls on uninitialized SBUF so
            # the PE clock-gate opens before the first real matmul arrives
            scratch = const_pool.tile([128, CHUNK], f16)
            nc.vector.memset(scratch[:], 0.0)
            ps_warm = warm_pool.tile([128, CHUNK], f32, tag="pswarm", name="pswarm")
            for _ in range(9):
                nc.tensor.matmul(ps_warm[:], scratch[:, 0:128], scratch[:],
                                 start=True, stop=True, skip_group_check=True)

            # per-block diagonal stationaries, built on device (emitted inside
            # the block loop so the pipeline starts immediately)
            dsilu = const_pool.tile([128, EB * 128], f32r)
            dspl = const_pool.tile([128, EB * 4 * 128], f16)

            def feature_ops(xv, n, tagsuf):
                """xv: [128, n, B_SHARD] input view -> 5 feature tiles."""
                pool = feat0_pool if tagsuf else feat_pool
                silu_t = pool.tile([128, n, B_SHARD], f32r, tag="silu" + tagsuf,
                                        name=f"silu{tagsuf}")
                nc.scalar.activation(silu_t[:], xv, AF.Silu)
                xc_t = pool.tile([128, n, B_SHARD], f16, tag="xc" + tagsuf,
                                      name=f"xc{tagsuf}")
                nc.vector.tensor_scalar(xc_t[:], xv, 1.0, -1.0, OP.min, OP.max)
                r1_t = pool.tile([128, n, B_SHARD], f16, tag="r1" + tagsuf,
                                      name=f"r1{tagsuf}")
                nc.vector.tensor_scalar(r1_t[:], xc_t[:], 0.5, 0.0, OP.add, OP.max)
                r2_t = pool.tile([128, n, B_SHARD], f16, tag="r2" + tagsuf,
                                      name=f"r2{tagsuf}")
                nc.vector.tensor_scalar_max(r2_t[:], xc_t[:], 0.0)
                r3_t = pool.tile([128, n, B_SHARD], f16, tag="r3" + tagsuf,
                                      name=f"r3{tagsuf}")
                nc.vector.tensor_scalar(r3_t[:], xc_t[:], -0.5, 0.0, OP.add, OP.max)
                return silu_t, xc_t, r1_t, r2_t, r3_t

            def block_matmuls(e, feats, hh, yo):
                """Build diags for block e, run the 10 matmuls, evacuate."""
                silu_t, xc_t, r1_t, r2_t, r3_t = feats
                ds = dsilu[:, e * 128:(e + 1) * 128]
                nc.vector.tensor_scalar_mul(ds, ident32, wv[:, e * NF:e * NF + 1])
                for j in range(4):
                    nc.vector.tensor_scalar_mul(
                        dspl[:, (e * 4 + j) * 128:(e * 4 + j + 1) * 128],
                        ident16[:], wv[:, e * NF + 1 + j:e * NF + 2 + j])
                ps = psum_pool.tile([128, B_SHARD], f32, tag="ps", name=f"ps_{e}")

                # xc is ready before silu (clip is cheaper than the ACT pass),
                # so start each block's accumulation with the spline features
                # and finish with silu
                def block_chunk(ts):
                    for j, ft in enumerate((xc_t, r1_t, r2_t, r3_t)):
                        for t in ts:
                            nc.tensor.matmul(ps[:, t * CHUNK:(t + 1) * CHUNK],
                                             dspl[:, (e * 4 + j) * 128:(e * 4 + j + 1) * 128],
                                             ft[:, hh, t * CHUNK:(t + 1) * CHUNK],
                                             start=(j == 0), stop=False,
                                             skip_group_check=True)
                    for t in ts:
                        nc.tensor.matmul(ps[:, t * CHUNK:(t + 1) * CHUNK], ds,
                                         silu_t[:, hh, t * CHUNK:(t + 1) * CHUNK],
                                         start=False, stop=True, skip_group_check=True)

                if e < EB - 1:
                    block_chunk((0, 1))
                    nc.scalar.activation(yo[:, e % 2, :], ps[:], AF.Identity,
                                         bias=cp[:, AOFF + e:AOFF + e + 1], scale=1.0)
                else:
                    # last block: per-chunk pipeline on VectorE for a short tail
                    for t in range(2):
                        block_chunk((t,))
                        nc.vector.tensor_scalar_add(
                            yo[:, e % 2, t * CHUNK:(t + 1) * CHUNK],
                            ps[:, t * CHUNK:(t + 1) * CHUNK],
                            cp[:, AOFF + e:AOFF + e + 1])

            for ep in range(EB // 2):
                if ep == 0:
                    # first pair: per-block DMAs and per-block features so
                    # compute starts as soon as 512 KB has landed
                    yo = yout_pool.tile([128, 2, B_SHARD], f16, tag="yo", name="yo_p0")
                    for h in range(2):
                        x_t = xin_pool.tile([128, 1, B_SHARD], f32, tag=f"x0{h}",
                                            name=f"x0{h}")
                        nc.sync.dma_start(x_t[:], xt3[:, h:h + 1, :])
                        feats = feature_ops(x_t[:], 1, f"0{h}")
                        block_matmuls(h, feats, 0, yo)
                else:
                    x_t = xin_pool.tile([128, 2, B_SHARD], f32, tag="x",
                                        name=f"x_p{ep}")
                    nc.sync.dma_start(x_t[:], xt3[:, 2 * ep:2 * ep + 2, :])
                    feats = feature_ops(x_t[:], 2, "")
                    yo = yout_pool.tile([128, 2, B_SHARD], f16, tag="yo",
                                        name=f"yo_p{ep}")
                    for h in range(2):
                        block_matmuls(2 * ep + h, feats, h, yo)
                if ep == EB // 2 - 1:
                    # split the last stores across two queues for a short tail
                    nc.gpsimd.dma_start(yt3[:, 2 * ep:2 * ep + 1, :], yo[:, 0:1, :])
                    nc.gpsimd.dma_start(yt3[:, 2 * ep + 1:2 * ep + 2, 0:CHUNK],
                                        yo[:, 1:2, 0:CHUNK])
                    nc.sync.dma_start(yt3[:, 2 * ep + 1:2 * ep + 2, CHUNK:B_SHARD],
                                      yo[:, 1:2, CHUNK:B_SHARD])
                elif ep % 2 == 0:
                    nc.gpsimd.dma_start(yt3[:, 2 * ep:2 * ep + 2, :], yo[:])
                else:
                    nc.sync.dma_start(yt3[:, 2 * ep:2 * ep + 2, :], yo[:])
    nc.compile()
    return nc


def _host_prep(X, coeffs, W, b):
    c = coeffs.astype(np.float64)
    W = W.astype(np.float64)
    b = b.astype(np.float64)
    m = 2.0 * (c[:, 1:] - c[:, :-1])            # [O, 4] slopes per unit xc
    w1 = W[:, 1]
    aprime = w1 * (c[:, 0] + m[:, 0]) + b        # const term (incl. m0*(xc+1) fold)
    bprime = w1 * m[:, 0]
    g = w1[:, None] * (m[:, 1:] - m[:, :-1])     # [O, 3] relu weights at s=-0.5,0,0.5
    wvec = np.stack([W[:, 0], bprime, g[:, 0], g[:, 1], g[:, 2]], axis=1)  # [O, 5]

    cpack = np.zeros((128, CCOLS), dtype=np.float32)
    cpack[:, 0:128] = np.eye(128, dtype=np.float32)
    for e in range(EB):
        for f in range(NF):
            cpack[:, WOFF + e * NF + f] = wvec[e * 128:(e + 1) * 128, f].astype(np.float32)
        cpack[:, AOFF + e] = aprime[e * 128:(e + 1) * 128].astype(np.float32)
    return cpack


def kernel(X, coeffs, W, b):
    global _nc_cache
    if _nc_cache is None:
        _nc_cache = _build()
    nc = _nc_cache

    cpack = _host_prep(X, coeffs, W, b)
    in_maps = []
    for c in range(N_CORES):
        xt_shard = np.ascontiguousarray(X[c * B_SHARD:(c + 1) * B_SHARD, :].T)
        in_maps.append({"xt": xt_shard, "cpack": cpack})

    res = run_bass_kernel_spmd(nc, in_maps, core_ids=list(range(N_CORES)))
    Y = np.empty((B, OUT_F), dtype=np.float32)
    for c in range(N_CORES):
        Y[c * B_SHARD:(c + 1) * B_SHARD, :] = res.results[c]["yt"].T.astype(np.float32)
    return Y
